# revision 17
# baseline (speedup 1.0000x reference)
"""Expected Calibration Error (ECE) kernel for Trainium2, 8 NeuronCores.

Problem: inputs [2e6, 128] f32 row-probabilities, targets [2e6] int64.
  conf_i = max_c inputs[i, c];  pred_i = argmax_c inputs[i, c]
  bin_i  = bucketize(conf_i, linspace(0, 1, 11), right=True) - 1
  ECE    = sum_b |corr_sum[b] - conf_sum[b]| / N

Strategy (data-parallel over rows, 250k rows per core):
  One custom fused DVE op per 128-row tile computes, per partition p
  (one row), streaming its 128 class probs v[c]:
      key[c] = round_to_mult_of_4(v[c] * 2^29) + (c == target_p)
      K[p]   = max(1, max_c key[c])
  The magic-number trick ((x + 2^25) - 2^25) rounds x < 2^24 to the nearest
  multiple of 4 exactly in fp32, and the +1 "target hit" bit is exact, so
      correct[p] = K - 4*rint(K/4)  in {0, 1}   (did the target attain the max)
      S4[p]      = K - correct[p]   = conf quantized to 2^-27, times 2^29.
  This is ONE DVE pass over the data (the memory-bound minimum).

  Keys are decoded in chunks on the (otherwise idle) GPSIMD engine into
  (S4, correct) pairs and cumulative >=-edge indicators G_b = [S4 >= e_b*2^29],
  then a tiny TensorE matmul per 128-row tile accumulates PSUM[2, 10]:
      out[0, b] = sum_i S4_i * G_b(i)      (scaled cumulative conf sums)
      out[1, b] = sum_i correct_i * G_b(i) (cumulative correct counts)
  All of that overlaps the DVE main loop.  Host finishes: per-bin values by
  differencing, |.| sum, / N.

Sharding: rows split evenly, 250,000 per core = 122 supertiles x 16 tiles
(p-major contiguous DMA) + 1 plain tile + 1 partial 16-row tile.
"""

import numpy as np

N = 2_000_000
C = 128
NCORES = 8
ROWS = N // NCORES            # 250_000
NST = 122                     # supertiles of 16 tiles (2048 rows each)
ST_ROWS = 128 * 16            # 2048
NT_MAIN = NST * 16            # 1952 tiles via supertiles
# tile 1952: 128 rows; tile 1953: 16 rows (partial)
NTG = NT_MAIN + 2             # 1954 key columns
PARTIAL_ROWS = ROWS - NST * ST_ROWS - 128  # 16

# key columns per decode/matmul chunk; smaller at the end to shrink the
# serial tail after the last custom op
CHUNK_SIZES = [256] * 7 + [60, 51, 51]
assert sum(CHUNK_SIZES) == NTG
CHUNK_STARTS = [sum(CHUNK_SIZES[:i]) for i in range(len(CHUNK_SIZES))]
NCHUNKS = len(CHUNK_SIZES)

SCALE_BITS = 29
SCALE = float(2 ** SCALE_BITS)
MAGIC = float(2 ** 25)
DEC_MAGIC = float(2 ** 23)

_EDGES_F32 = np.linspace(0.0, 1.0, 11).astype(np.float32)  # matches jnp.linspace
EDGES_SCALED = [float(_EDGES_F32[b]) * SCALE for b in range(10)]

_f32 = np.float32


def _ece_pack_ref(in0, in1, c0, c1, c2):
    P = in0.shape[0]
    x = in0.astype(np.float32).reshape(P, -1)
    n = x.shape[1]
    s = (x * _f32(c2)).astype(np.float32)
    r = ((s + _f32(c1)).astype(np.float32) - _f32(c1)).astype(np.float32)
    idx = np.arange(n, dtype=np.float32)[None, :]
    tgt = np.asarray(in1, np.float32).reshape(P, -1)[:, 0:1]
    key = (r + (idx == tgt).astype(np.float32)).astype(np.float32)
    acc = np.maximum(key.max(axis=1, keepdims=True), 1.0).astype(np.float32)
    return key, acc


def _register_op():
    from concourse.dve_ops import (
        DveOp,
        OPS,
        CUSTOM_DVE_SPECS,
        _SUB_OPCODE_FOR_NAME,
        _CUSTOM_DVE_ROW_BASE,
    )
    from concourse.dve_spec import (
        Spec,
        Src0,
        C1,
        C2,
        C3,
        One,
        eq,
        maxx,
        lower,
        Idx,
        _spill_c3_to_src1,
    )
    from concourse.dve_uop import DveOpSpec

    name = "ECE_PACK_ANT"
    if name in _SUB_OPCODE_FOR_NAME:
        return next(op for op in OPS if op.name == name)

    # target rides in1, read once at element 0 via the C3->Latch(Src1) spill
    body = ((Src0 * C2 + C1) - C1) + eq(Idx, C3)
    spec = Spec(
        body=_spill_c3_to_src1(body),
        accum=maxx,
        accum_init=One,
        reference=_ece_pack_ref,
    )

    row = _CUSTOM_DVE_ROW_BASE + len(OPS)
    assert row < 0x20
    _SUB_OPCODE_FOR_NAME[name] = row
    shas = {}
    for ver in ("v3", "v4"):
        try:
            uops = lower(spec, ver=ver)
            shas[ver] = DveOpSpec(
                name=name, opcode=row, uops=uops, rd1_en=True
            ).sha(ver)
        except Exception:
            pass
    op = DveOp(name, spec, subdim=False, uops_sha=shas)
    OPS.append(op)
    CUSTOM_DVE_SPECS[name] = spec
    return op


_NC_CACHE = None


def _build_bass():
    global _NC_CACHE
    if _NC_CACHE is not None:
        return _NC_CACHE

    import concourse.bacc as bacc
    import concourse.tile as tile
    from concourse import mybir

    ece_op = _register_op()

    nc = bacc.Bacc()
    f32 = mybir.dt.float32
    x = nc.dram_tensor("x", [ROWS, C], f32, kind="ExternalInput")
    tg = nc.dram_tensor("tg", [128, NTG], f32, kind="ExternalInput")
    out = nc.dram_tensor("out", [2, 10], f32, kind="ExternalOutput")
    # last two chunks use one diagonal-batched matmul each
    LCS = CHUNK_SIZES[-2:]
    out2a = nc.dram_tensor("out2a", [2 * LCS[0], 10 * LCS[0]], f32, kind="ExternalOutput")
    out2b = nc.dram_tensor("out2b", [2 * LCS[1], 10 * LCS[1]], f32, kind="ExternalOutput")

    with tile.TileContext(nc) as tc:
        with (
            tc.tile_pool(name="persist", bufs=1) as persist,
            tc.tile_pool(name="inbuf", bufs=6) as inbuf,
            tc.tile_pool(name="tailbuf", bufs=1) as tailbuf,
            tc.tile_pool(name="scratch", bufs=8) as scratch,
            tc.tile_pool(name="decbuf", bufs=3) as decbuf,
            tc.tile_pool(name="psum", bufs=1, space="PSUM") as psumpool,
        ):
            tg_tiles = [
                persist.tile(
                    [128, CHUNK_SIZES[c]], f32, name=f"tgt{c}", tag=f"tgt{c}"
                )
                for c in range(NCHUNKS)
            ]
            nc.gpsimd.dma_start(
                out=tg_tiles[0][:], in_=tg[:][:, : CHUNK_SIZES[0]]
            )

            # one key tile per chunk so chunk decode only depends on its
            # own chunk's writers
            key_tiles = [
                persist.tile(
                    [128, CHUNK_SIZES[c]], f32, name=f"key{c}", tag=f"key{c}"
                )
                for c in range(NCHUNKS)
            ]
            # partial-tile column: partitions 16.. are never written
            nc.vector.memset(key_tiles[-1][:], 0.0)

            # per-edge biases for the ScalarE Sign ops ([P,1] APs; arbitrary
            # float literals have no const AP)
            edge_bias = persist.tile([128, 10], f32)
            for b in range(1, 10):
                nc.vector.memset(edge_bias[:, b : b + 1], -EDGES_SCALED[b])

            psum = psumpool.tile([2, 10], f32)
            psum2 = {
                NCHUNKS - 2: psumpool.tile(
                    [2 * LCS[0], 10 * LCS[0]], f32, name="ps2a", tag="ps2a"
                ),
                NCHUNKS - 1: psumpool.tile(
                    [2 * LCS[1], 10 * LCS[1]], f32, name="ps2b", tag="ps2b"
                ),
            }

            x_ap = x[:]
            xr = x_ap[: NST * ST_ROWS, :].rearrange(
                "(s p k) c -> s p k c", s=NST, p=128, k=16
            )

            import bisect

            def emit_tile_op(in0_ap, j, nparts=128):
                c = bisect.bisect_right(CHUNK_STARTS, j) - 1
                l = j - CHUNK_STARTS[c]
                dump = scratch.tile([128, C], f32, name="dump", tag="dump")
                nc.vector._custom_dve(
                    ece_op,
                    out=dump[:nparts, :],
                    in0=in0_ap,
                    in1=tg_tiles[c][:nparts, l : l + 1],
                    s1=MAGIC,
                    imm2=SCALE,
                    accum_out=key_tiles[c][:nparts, l : l + 1],
                )

            def emit_chunk_epilogue(c):
                ncols = CHUNK_SIZES[c]
                kt = key_tiles[c]
                if c >= NCHUNKS - 2:
                    # exact-size contiguous tiles so the batched matmul can
                    # flatten them to a single free dim
                    cc = decbuf.tile([128, 2, ncols], f32, name=f"cc2_{c}", tag=f"cc2_{c}", bufs=1)
                    g = decbuf.tile([128, 10, ncols], f32, name=f"g2_{c}", tag=f"g2_{c}", bufs=1)
                else:
                    cc = decbuf.tile([128, 2, 256], f32, name="cc", tag="cc")
                    g = decbuf.tile([128, 10, 256], f32, name="g", tag="g")
                t1 = decbuf.tile([128, 256], f32, name="t1", tag="t1")
                # t1 = 4*rint(K/4) via two exact fp32 affines on ScalarE:
                #   a = K*0.25 + 2^23 ;  t1 = a*4 - 2^25
                nc.scalar.activation(
                    out=t1[:, :ncols],
                    in_=kt[:, :ncols],
                    func=mybir.ActivationFunctionType.Copy,
                    bias=DEC_MAGIC,
                    scale=0.25,
                )
                nc.scalar.activation(
                    out=t1[:, :ncols],
                    in_=t1[:, :ncols],
                    func=mybir.ActivationFunctionType.Copy,
                    bias=-float(2 ** 25),
                    scale=4.0,
                )
                nc.vector.tensor_tensor(
                    out=cc[:, 1, :ncols],
                    in0=kt[:, :ncols],
                    in1=t1[:, :ncols],
                    op=mybir.AluOpType.subtract,
                )
                nc.vector.tensor_tensor(
                    out=cc[:, 0, :ncols],
                    in0=kt[:, :ncols],
                    in1=cc[:, 1, :ncols],
                    op=mybir.AluOpType.subtract,
                )
                # G_0 = 1 always; G_b (b>=1) as sign(S4 - E_b) in {-1, +1}
                # (S4 == E_b impossible: S4 is a multiple of 4, E_b is not an
                # integer for b>=1).  Host recovers [S4 >= E_b] sums via
                # (S_b + S_0) / 2.
                nc.scalar.activation(
                    out=g[:, 0, :ncols],
                    in_=kt[:, :ncols],
                    func=mybir.ActivationFunctionType.Copy,
                    bias=1.0,
                    scale=0.0,
                )
                for b in range(1, 10):
                    nc.scalar.activation(
                        out=g[:, b, :ncols],
                        in_=cc[:, 0, :ncols],
                        func=mybir.ActivationFunctionType.Sign,
                        bias=edge_bias[:, b : b + 1],
                        scale=1.0,
                    )
                if c >= NCHUNKS - 2:
                    # single diagonal-batched matmul; host extracts the
                    # [2,10] diagonal blocks of the result
                    nc.tensor.matmul(
                        psum2[c][:],
                        lhsT=cc[:].rearrange("p a b -> p (a b)"),
                        rhs=g[:].rearrange("p a b -> p (a b)"),
                        start=True,
                        stop=True,
                    )
                else:
                    for l in range(ncols):
                        j = CHUNK_STARTS[c] + l
                        nc.tensor.matmul(
                            psum[:],
                            lhsT=cc[:, :, l],
                            rhs=g[:, :, l],
                            start=(j == 0),
                            stop=(j == CHUNK_STARTS[-2] - 1),
                        )

            # supertile 0 split into quarter-DMAs so compute starts early
            boots = []
            for qi in range(4):
                q = inbuf.tile(
                    [128, 4, C], f32, name=f"q{qi}", tag=f"q{qi}", bufs=1
                )
                nc.sync.dma_start(out=q[:], in_=xr[0][:, 4 * qi : 4 * qi + 4, :])
                boots.append(q)

            # tail full tile (rows 249856:249984) -> column 1952
            xt2 = tailbuf.tile([128, C], f32)
            nc.sync.dma_start(
                out=xt2[:], in_=x_ap[NST * ST_ROWS : NST * ST_ROWS + 128, :]
            )
            # partial tile (16 rows, 249984:250000) -> column 1953
            xt3 = tailbuf.tile([PARTIAL_ROWS, C], f32)
            nc.sync.dma_start(out=xt3[:], in_=x_ap[NST * ST_ROWS + 128 :, :])

            for c in range(1, NCHUNKS):
                a = CHUNK_STARTS[c]
                nc.sync.dma_start(
                    out=tg_tiles[c][:], in_=tg[:][:, a : a + CHUNK_SIZES[c]]
                )

            for k in range(16):
                emit_tile_op(boots[k // 4][:, k % 4, :], k)
            emit_tile_op(xt2[:], NT_MAIN)
            emit_tile_op(xt3[:], NT_MAIN + 1, nparts=PARTIAL_ROWS)

            fired = [0]
            st_tiles = {}

            def load_st(si):
                t = inbuf.tile([128, 16, C], f32, name="xt", tag="xt")
                nc.sync.dma_start(out=t[:], in_=xr[si])
                st_tiles[si] = t

            for si in (1, 2, 3, 4):
                load_st(si)
            for s in range(1, NST):
                xt = st_tiles.pop(s)
                if s + 4 < NST:
                    load_st(s + 4)
                for k in range(16):
                    emit_tile_op(xt[:, k, :], s * 16 + k)
                done = (s + 1) * 16
                while (
                    fired[0] < NCHUNKS - 1
                    and CHUNK_STARTS[fired[0]] + CHUNK_SIZES[fired[0]] <= done
                ):
                    emit_chunk_epilogue(fired[0])
                    fired[0] += 1

            while fired[0] < NCHUNKS:
                emit_chunk_epilogue(fired[0])
                fired[0] += 1

            res = persist.tile([2, 10], f32)
            nc.vector.tensor_copy(out=res[:], in_=psum[:])
            nc.sync.dma_start(out=out[:], in_=res[:])
            res2a = persist.tile([2 * LCS[0], 10 * LCS[0]], f32)
            nc.vector.tensor_copy(out=res2a[:], in_=psum2[NCHUNKS - 2][:])
            nc.sync.dma_start(out=out2a[:], in_=res2a[:])
            res2b = persist.tile([2 * LCS[1], 10 * LCS[1]], f32)
            nc.vector.tensor_copy(out=res2b[:], in_=psum2[NCHUNKS - 1][:])
            nc.sync.dma_start(out=out2b[:], in_=res2b[:])

    nc.finalize()
    _NC_CACHE = nc
    return nc


def _prep_targets(t_loc: np.ndarray) -> np.ndarray:
    """[ROWS] int targets -> [128, NTG] f32, laid out per tile."""
    s0 = t_loc.astype(np.float32)
    tg = np.zeros((128, NTG), dtype=np.float32)
    main = s0[: NST * ST_ROWS].reshape(NST, 128, 16)
    tg[:, :NT_MAIN] = main.transpose(1, 0, 2).reshape(128, NT_MAIN)
    tg[:, NT_MAIN] = s0[NST * ST_ROWS : NST * ST_ROWS + 128]
    tg[:PARTIAL_ROWS, NT_MAIN + 1] = s0[NST * ST_ROWS + 128 :]
    return tg


def _run(inputs: np.ndarray, targets: np.ndarray, trace: bool = False):
    from concourse.bass_utils import run_bass_kernel_spmd

    nc = _build_bass()

    inputs = np.ascontiguousarray(inputs, dtype=np.float32)
    targets = np.asarray(targets)

    in_maps = []
    for k in range(NCORES):
        lo = k * ROWS
        xs = inputs[lo : lo + ROWS]
        tgc = _prep_targets(targets[lo : lo + ROWS])
        in_maps.append({"x": xs, "tg": tgc})

    last_err = None
    for _attempt in range(3):
        try:
            r = run_bass_kernel_spmd(
                nc, in_maps, core_ids=list(range(NCORES)), trace=trace
            )
            break
        except Exception as e:  # transient NRT_EXEC_UNIT_UNRECOVERABLE on cold device
            last_err = e
    else:
        raise last_err
    return r


def _combine(results) -> np.ndarray:
    S = np.zeros((2, 10), dtype=np.float64)
    for r in results:
        S += r["out"].astype(np.float64)
        for nm, lc in (("out2a", CHUNK_SIZES[-2]), ("out2b", CHUNK_SIZES[-1])):
            o2 = r[nm].astype(np.float64).reshape(2, lc, 10, lc)
            S += np.einsum("ajbj->ab", o2)
    # G columns b>=1 were sign in {-1,+1}: [x >= E_b] = (sign + 1) / 2
    S[:, 1:] = (S[:, 1:] + S[:, 0:1]) / 2.0
    Sc = S[0] / SCALE
    Sk = S[1]
    conf_sum = Sc - np.append(Sc[1:], 0.0)
    corr_sum = Sk - np.append(Sk[1:], 0.0)
    ece = np.abs(corr_sum - conf_sum).sum() / N
    return np.asarray(ece, dtype=np.float32)


def kernel(inputs: np.ndarray, targets: np.ndarray) -> np.ndarray:
    r = _run(inputs, targets, trace=False)
    return _combine(r.results)
